# revision 38
# baseline (speedup 1.0000x reference)
"""AutoRec forward kernel for Trainium2, 8-core SPMD.

Math (see reference):
    agg = segment_sum(r[:,None] * v[cols], rows, m)     # sparse (m,n) @ v
    h   = sigmoid(agg + mu)                             # (M, D)
    s   = sum(h[i] * w[j])                              # global scalar over E pairs
    out = s + b[j]                                      # (E,)

Device strategy (per core, users sharded):
  Each core owns RPC = 6272 rows (users). Both heavy stages are instances of
  one primitive: "gather rows from a replicated table, weight them, and
  segment-sum into a local per-row accumulator":
    phase 1: table=v (bf16), weights=r,     rows=ij[0], cols=ij[1] -> aggT
    phase 2: table=w (f32),  weights=1.0,   rows=i,     cols=j     -> aT
          (sum_e h[i_e] * w[j_e] = sum_u h[u] . A[u],  A[u] = sum_{i_e=u} w[j_e])
  The segment-sum runs on the tensor engine: for each chunk of 128 edges the
  gathered rows (f16, SWDGE dma_gather) form the stationary operand
  [128e, 128d]; a host-built one-hot P[e, wrow] = weight_e *
  (local_row_e == wrow), streamed via HWDGE at fp8e4, is the moving operand;
  psum accumulates aggT[d, wrow] over a 128-row window. Tables are split in
  two 25000-row halves (dma_gather indices are int16). Groups are ordered
  window-major (snake over halves) so each window's psum spans its two
  adjacent half-groups; at window close, phase 1 folds sigmoid(psum + mu)
  into the h accumulator on ACT straight from psum, and phase 2 folds
  s_cols[:, w] = sum_u h_win * psum_win on DVE - no second accumulator and
  no serial tail. Rows are remapped to (core, window, slot) by a 4-D LPT
  (phase x table-half degree vectors) so per-group counts are near-equal
  across cores, tightening the shared SPMD chunk schedule (all 8 cores run
  one program; schedule = max count per group over cores). Full wrapped idx
  arrays are preloaded into SBUF once, killing per-call idx DMAs. The host
  sums the 8 s_out partials and broadcasts s + b[j] (trivial O(E) numpy).
"""

import math
from dataclasses import dataclass, field

import ml_dtypes
import numpy as np

# ---------------------------------------------------------------- config

CHUNK = 128  # edges per matmul (contraction = partition dim)
IDX_WRAP = 16  # dma_gather index wrap


@dataclass
class Cfg:
    M: int = 50000          # users (rows of spmm)
    dma_scratch: int = 16384  # SWDGE descriptor carveout (bytes)
    N: int = 50000          # items (table rows)
    D: int = 128            # feature dim (must be 128)
    ncores: int = 8
    rpc: int = 6272         # rows per core (multiple of window)
    window: int = 128       # psum row-window
    half: int = 25000       # table split (int16 index limit)
    call_chunks: int = 8    # chunks per dma_gather call (HW SWDGE ring caps ~1024 idxs/call)
    p1dt: str = "f16"       # value dtype of phase-1 gathers / one-hot
    p2dt: str = "f16"       # value dtype of phase-2 gathers / one-hot
    psdt: str = ""          # P stream dtype override ("" = same as gathers)
    wmajor: bool = False    # window-major group order + fused per-window fold
    ttb: int = 512          # block size of the final fused mul-reduce
    queues: int = 4         # SWDGE queues to round-robin gather calls over
    balance: bool = False   # degree-balanced row->(core,window,slot) remap
    host_p: bool = True     # precompute one-hot P on host, stream via HWDGE
    p_dve_frac: float = 0.0  # fraction of chunks whose P is DVE-built instead
    swap_ops: bool = False  # P as stationary (fp8 LS), gathered tile as moving
    gbufs: int = 8          # gather-tile pool bufs
    pbufs: int = 10         # one-hot P pool bufs
    ibufs: int = 10         # idx pool bufs

    @property
    def nwin(self):
        return self.rpc // self.window

    def __post_init__(self):
        assert self.rpc % self.window == 0
        assert self.rpc * self.ncores >= self.M
        assert self.N <= 2 * self.half and self.half <= 32767
        assert self.D == 128


FULL = Cfg(wmajor=True, host_p=True, balance=True, call_chunks=8,
           dma_scratch=49152, gbufs=8, ibufs=2, pbufs=12, psdt="f8e4",
           swap_ops=True)

# ---------------------------------------------------------------- host plan


@dataclass
class PhasePlan:
    groups: list          # [(hf, win, n_chunks)] in stream order (hf-major)
    calls: list           # [(hf, chunk_start, n_chunks)]
    total_chunks: int
    # per-core packed arrays
    idx_dram: list        # [ncores] int16 [128, total_chunks*8]
    wgt_dram: list        # [ncores] [128, total_chunks]
    rl_dram: list         # [ncores] [128, total_chunks]
    p_dram: list = None   # [ncores] pdt [128, total_chunks*W] host one-hot


def _wrap_idxs(ii: np.ndarray) -> np.ndarray:
    """[n] -> [128, n/16] wrapped (t -> (t%16, t//16)), replicated x8."""
    n = len(ii)
    a = ii.reshape(n // IDX_WRAP, IDX_WRAP).T
    return np.tile(a, (8, 1))


def balance_rows(cfg: Cfg, rows1, cols1, rows2, cols2) -> np.ndarray:
    """4-D LPT: pack rows into (core, window) buckets of 128 slots balancing
    each row's (phase x table-half) degree vector, so per-(half,win) counts
    are near-equal across cores -> tight shared chunk schedule.
    Returns rowmap [M]: row -> virtual row id (core*rpc + win*128 + slot)."""
    nb = cfg.ncores * cfg.nwin
    deg = np.zeros((cfg.M, 4), np.int64)
    for k, (rr, cc) in enumerate(((rows1, cols1), (rows2, cols2))):
        rr = np.asarray(rr, np.int64)
        hf = (np.asarray(cc, np.int64) >= cfg.half).astype(np.int64)
        np.add.at(deg, (rr, 2 * k + hf), 1)
    order = np.argsort(-deg.sum(1), kind="stable")
    load = np.zeros((nb, 4), np.float64)
    fill = np.zeros(nb, np.int64)
    full_pen = np.zeros(nb, np.float64)
    rowmap = np.zeros(cfg.M, np.int64)
    for row in order:
        cost = (load + deg[row]).max(axis=1) + full_pen
        b = int(np.argmin(cost))
        rowmap[row] = b * cfg.window + fill[b]
        fill[b] += 1
        load[b] += deg[row]
        if fill[b] >= cfg.window:
            full_pen[b] = np.inf
    return rowmap


def plan_phase(cfg: Cfg, rows, cols, wgts, pnp=None) -> PhasePlan:
    rows = np.asarray(rows, np.int64)
    cols = np.asarray(cols, np.int64)
    wgts = np.asarray(wgts, np.float32)
    nwin, ncores, Wd = cfg.nwin, cfg.ncores, cfg.window

    core = rows // cfg.rpc
    local = rows - core * cfg.rpc
    win = local // Wd
    rl = (local - win * Wd).astype(np.float32)
    hf = (cols >= cfg.half).astype(np.int64)
    idx16 = (cols - hf * cfg.half).astype(np.int16)

    key = (core * 2 + hf) * nwin + win
    counts = np.bincount(key, minlength=ncores * 2 * nwin).reshape(ncores, 2, nwin)
    nch = -(-counts.max(axis=0) // CHUNK)  # [2, nwin] ceil
    groups = []
    gbase = np.zeros((2, nwin), np.int64)
    acc = 0
    if cfg.wmajor:
        # window-major, snake over halves so adjacent same-half runs merge
        order = []
        for w in range(nwin):
            hs = (0, 1) if w % 2 == 0 else (1, 0)
            for h in hs:
                order.append((h, w))
    else:
        order = [(h, w) for h in range(2) for w in range(nwin)]
    for h, w in order:
        n = int(nch[h, w])
        if n == 0:
            continue
        groups.append((h, w, n))
        gbase[h, w] = acc
        acc += n
    total_chunks = acc

    # gather calls: split maximal same-half chunk runs into spans of
    # call_chunks (a call reads one table half)
    calls = []
    runs = []
    for g, (h, w, n) in enumerate(groups):
        if runs and runs[-1][0] == h:
            runs[-1][2] += n
        else:
            runs.append([h, sum(nn for (_, _, nn) in groups[:g]), n])
    for h, start, n in runs:
        off = start
        while off < start + n:
            k = min(cfg.call_chunks, start + n - off)
            calls.append((h, off, k))
            off += k
    assert sum(n for (_, _, n) in calls) == total_chunks

    idx_l, wgt_l, rl_l, p_l = [], [], [], []
    for c in range(ncores):
        mask = core == c
        eh, ew = hf[mask], win[mask]
        erl, ei, ewgt = rl[mask], idx16[mask], wgts[mask]
        order = np.lexsort((ew, eh))
        eh, ew, erl, ei, ewgt = (a[order] for a in (eh, ew, erl, ei, ewgt))
        gid = eh * nwin + ew
        # rank within each (hf,win) run of the sorted list
        if len(gid):
            first = np.r_[True, gid[1:] != gid[:-1]]
            run_start = np.maximum.accumulate(np.where(first, np.arange(len(gid)), 0))
            rank = np.arange(len(gid)) - run_start
        else:
            rank = np.zeros(0, np.int64)
        pos = gbase[eh, ew] * CHUNK + rank
        idx_full = np.zeros(total_chunks * CHUNK, np.int16)
        wgt_full = np.zeros(total_chunks * CHUNK, np.float32)
        rl_full = np.zeros(total_chunks * CHUNK, np.float32)
        idx_full[pos] = ei
        wgt_full[pos] = ewgt
        rl_full[pos] = erl

        # wrap idx per call
        parts = []
        for (_h, c0, n) in calls:
            parts.append(_wrap_idxs(idx_full[c0 * CHUNK:(c0 + n) * CHUNK]))
        idx_l.append(np.concatenate(parts, axis=1))
        wgt_l.append(wgt_full.reshape(-1, CHUNK).T.copy())
        rl_l.append(rl_full.reshape(-1, CHUNK).T.copy())
        if pnp is not None:
            # host one-hot: P[p, chunk, x] = wgt * (rl == x), edge = chunk*128+p
            parr = np.zeros((CHUNK, total_chunks, Wd), pnp)
            epos = np.arange(total_chunks * CHUNK)
            parr[epos % CHUNK, epos // CHUNK, rl_full.astype(np.int64)] = wgt_full
            p_l.append(np.ascontiguousarray(parr.reshape(CHUNK, -1)))

    return PhasePlan(groups, calls, total_chunks, idx_l, wgt_l, rl_l,
                     p_l if pnp is not None else None)


# ---------------------------------------------------------------- device build


def build_program(cfg: Cfg, ph1: PhasePlan, ph2: PhasePlan):
    import concourse.bacc as bacc
    import concourse.bass as bass
    import concourse.mybir as mybir
    import concourse.tile as tile

    f32 = mybir.dt.float32
    i16 = mybir.dt.int16
    DTMAP = {"f32": f32, "bf16": mybir.dt.bfloat16, "f16": mybir.dt.float16,
             "f8e4": mybir.dt.float8e4}
    p1dt, p2dt = DTMAP[cfg.p1dt], DTMAP[cfg.p2dt]
    psdt = DTMAP[cfg.psdt] if cfg.psdt else None
    P, Wd, RPC = 128, cfg.window, cfg.rpc
    n_hi = cfg.N - cfg.half

    nc = bacc.Bacc("TRN2", target_bir_lowering=False, debug=False,
                   dynamic_dma_scratch_size=cfg.dma_scratch,
                   num_swdge_queues=cfg.queues)

    v_lo = nc.dram_tensor("v_lo", [cfg.half, cfg.D], p1dt, kind="ExternalInput")
    v_hi = nc.dram_tensor("v_hi", [n_hi, cfg.D], p1dt, kind="ExternalInput")
    w_lo = nc.dram_tensor("w_lo", [cfg.half, cfg.D], p2dt, kind="ExternalInput")
    w_hi = nc.dram_tensor("w_hi", [n_hi, cfg.D], p2dt, kind="ExternalInput")
    mu_c = nc.dram_tensor("mu_col", [P, 1], f32, kind="ExternalInput")
    if cfg.swap_ops:
        mu_r = nc.dram_tensor("mu_row", [P, cfg.D], f32, kind="ExternalInput")

    need_build = (not cfg.host_p) or cfg.p_dve_frac > 0

    def phase_params(tag, pl: PhasePlan, pdt):
        idx = nc.dram_tensor(f"idx{tag}", [P, pl.total_chunks * 8], i16,
                             kind="ExternalInput")
        pd = wgt = rl = None
        if cfg.host_p:
            pd = nc.dram_tensor(f"p{tag}", [P, pl.total_chunks * Wd],
                                psdt or pdt, kind="ExternalInput")
        if need_build:
            wgt = nc.dram_tensor(f"wgt{tag}", [P, pl.total_chunks], f32,
                                 kind="ExternalInput")
            rl = nc.dram_tensor(f"rl{tag}", [P, pl.total_chunks], f32,
                                kind="ExternalInput")
        return idx, pd, wgt, rl

    idx1, p1, wgt1, rl1 = phase_params(1, ph1, p1dt)
    idx2, p2, wgt2, rl2 = phase_params(2, ph2, p2dt)
    s_out = nc.dram_tensor("s_out", [P, 1], f32, kind="ExternalOutput")

    with tile.TileContext(nc) as tc:
        with (
            tc.tile_pool(name="const", bufs=1) as cpool,
            tc.tile_pool(name="idxp", bufs=cfg.ibufs) as ipool,
            tc.tile_pool(name="g1", bufs=cfg.gbufs) as g1pool,
            tc.tile_pool(name="g2", bufs=cfg.gbufs) as g2pool,
            tc.tile_pool(name="pp", bufs=cfg.pbufs) as ppool,
            tc.tile_pool(name="ev", bufs=4) as evpool,
            tc.tile_pool(name="psum", bufs=8, space="PSUM") as pspool,
        ):
            # constants
            mu_t = cpool.tile([P, 1], f32, tag="mu")
            nc.sync.dma_start(mu_t[:], mu_c[:])
            if cfg.swap_ops:
                mu_rt = cpool.tile([P, cfg.D], f32, tag="mur")
                nc.sync.dma_start(mu_rt[:], mu_r[:])
            if need_build:
                iota1 = cpool.tile([P, Wd], p1dt, tag="iota1")
                iota2 = cpool.tile([P, Wd], p2dt, tag="iota2")
                nc.gpsimd.iota(iota1[:], pattern=[[1, Wd]], base=0,
                               channel_multiplier=0,
                               allow_small_or_imprecise_dtypes=True)
                nc.gpsimd.iota(iota2[:], pattern=[[1, Wd]], base=0,
                               channel_multiplier=0,
                               allow_small_or_imprecise_dtypes=True)
            else:
                iota1 = iota2 = None

            acc1 = cpool.tile([P, RPC], f32, tag="acc1")
            nc.vector.memset(acc1[:], 0.0)
            if not cfg.wmajor:
                acc2 = cpool.tile([P, RPC], f32, tag="acc2")
                nc.vector.memset(acc2[:], 0.0)
            else:
                acc2 = None

            if need_build:
                wg1_t = cpool.tile([P, ph1.total_chunks], f32, tag="wg1")
                rl1_t = cpool.tile([P, ph1.total_chunks], f32, tag="rl1")
                wg2_t = cpool.tile([P, ph2.total_chunks], f32, tag="wg2")
                rl2_t = cpool.tile([P, ph2.total_chunks], f32, tag="rl2")
                nc.sync.dma_start(wg1_t[:], wgt1[:])
                nc.sync.dma_start(rl1_t[:], rl1[:])
                nc.sync.dma_start(wg2_t[:], wgt2[:])
                nc.sync.dma_start(rl2_t[:], rl2[:])
            else:
                wg1_t = rl1_t = wg2_t = rl2_t = None

            qcount = [0]

            # preload full wrapped-idx arrays once (kills per-call idx DMAs)
            idx1_t = cpool.tile([P, ph1.total_chunks * 8], i16, tag="idx1t")
            idx2_t = cpool.tile([P, ph2.total_chunks * 8], i16, tag="idx2t")
            nc.sync.dma_start(idx1_t[:], idx1[:])
            nc.sync.dma_start(idx2_t[:], idx2[:])

            def run_phase(pl: PhasePlan, tabs, idx_t, p_dram, wg_t, rl_t,
                          fold, gpool, pdt, io_t):
                # map chunk id -> (win, first?, last?); first/last span the
                # whole window (wmajor: its adjacent half-groups) or one group
                chunk_group = {}
                spans = []  # (w, start_chunk, n_chunks)
                base = 0
                for (h, w, n) in pl.groups:
                    if cfg.wmajor and spans and spans[-1][0] == w:
                        spans[-1][2] += n
                    else:
                        spans.append([w, base, n])
                    base += n
                for w, s0, n in spans:
                    for k in range(n):
                        chunk_group[s0 + k] = (w, k == 0, k == n - 1)
                # consume calls in order, carrying the open psum group
                open_ps = None
                for (h, c0, n) in pl.calls:
                    gt = gpool.tile([P, cfg.call_chunks, cfg.D], pdt, tag="g")
                    nidx = n * CHUNK
                    nc.gpsimd.dma_gather(
                        gt[:, :n, :], tabs[h][:], idx_t[:, c0 * 8:(c0 + n) * 8],
                        num_idxs=nidx, num_idxs_reg=nidx, elem_size=cfg.D,
                        queue_num=qcount[0] % cfg.queues,
                    )
                    qcount[0] += 1
                    if not cfg.host_p:
                        k_dve = n
                    else:
                        k_dve = int(round(cfg.p_dve_frac * n))
                    if n > k_dve:
                        pc_t = ppool.tile([P, cfg.call_chunks * Wd],
                                          psdt or pdt, tag="pc")
                        nc.sync.dma_start(
                            pc_t[:, : (n - k_dve) * Wd],
                            p_dram[:, (c0 + k_dve) * Wd:(c0 + n) * Wd])
                    for k in range(n):
                        cid = c0 + k
                        w, first, last = chunk_group[cid]
                        if first:
                            open_ps = pspool.tile([P, Wd], mybir.dt.float32,
                                                  tag="ps")
                        if k >= k_dve:
                            p_ap = pc_t[:, (k - k_dve) * Wd:(k - k_dve + 1) * Wd]
                        else:
                            p_t = ppool.tile([P, Wd], pdt, tag="p")
                            nc.vector.tensor_scalar(
                                out=p_t[:], in0=io_t[:],
                                scalar1=rl_t[:, cid:cid + 1],
                                scalar2=wg_t[:, cid:cid + 1],
                                op0=mybir.AluOpType.is_equal,
                                op1=mybir.AluOpType.mult,
                            )
                            p_ap = p_t[:]
                        if cfg.swap_ops:
                            nc.tensor.matmul(open_ps[:], p_ap, gt[:, k, :],
                                             start=first, stop=last)
                        else:
                            nc.tensor.matmul(open_ps[:], gt[:, k, :], p_ap,
                                             start=first, stop=last)
                        if last:
                            fold(w, open_ps)

            if cfg.wmajor:
                s_cols = cpool.tile([P, cfg.nwin], f32, tag="scols")
                nc.vector.memset(s_cols[:], 0.0)

                def fold1(w, ps):
                    # h_win = sigmoid(agg_win + mu), psum -> acc1 slice
                    if cfg.swap_ops:
                        # [u, d] layout: mu varies along free dim; add the
                        # replicated mu row on DVE, then plain sigmoid on ACT
                        nc.vector.tensor_tensor(
                            out=ps[:], in0=ps[:], in1=mu_rt[:],
                            op=mybir.AluOpType.add)
                        nc.scalar.activation(
                            acc1[:, w * Wd:(w + 1) * Wd], ps[:],
                            mybir.ActivationFunctionType.Sigmoid,
                            bias=0.0, scale=1.0)
                    else:
                        nc.scalar.activation(
                            acc1[:, w * Wd:(w + 1) * Wd], ps[:],
                            mybir.ActivationFunctionType.Sigmoid,
                            bias=mu_t[:, 0:1], scale=1.0)

                def fold2(w, ps):
                    # s_cols[:, w] = sum_u h_win * A_win  (A straight from psum)
                    tmp = evpool.tile([P, Wd], f32, tag="tmp")
                    nc.vector.tensor_tensor(
                        out=tmp[:], in0=acc1[:, w * Wd:(w + 1) * Wd],
                        in1=ps[:], op=mybir.AluOpType.mult)
                    nc.vector.tensor_reduce(
                        s_cols[:, w:w + 1], tmp[:],
                        axis=mybir.AxisListType.X, op=mybir.AluOpType.add)

                run_phase(ph1, (v_lo, v_hi), idx1_t, p1, wg1_t, rl1_t,
                          fold1, g1pool, p1dt, iota1)
                run_phase(ph2, (w_lo, w_hi), idx2_t, p2, wg2_t, rl2_t,
                          fold2, g2pool, p2dt, iota2)
            else:
                def mkfold(acc):
                    def fold(w, ps):
                        sl = acc[:, w * Wd:(w + 1) * Wd]
                        nc.vector.tensor_tensor(
                            out=sl, in0=sl, in1=ps[:],
                            op=mybir.AluOpType.add)
                    return fold

                run_phase(ph1, (v_lo, v_hi), idx1_t, p1, wg1_t, rl1_t,
                          mkfold(acc1), g1pool, p1dt, iota1)
                run_phase(ph2, (w_lo, w_hi), idx2_t, p2, wg2_t, rl2_t,
                          mkfold(acc2), g2pool, p2dt, iota2)

                # h = sigmoid(aggT + mu)  (in place on acc1)
                nc.scalar.activation(acc1[:], acc1[:],
                                     mybir.ActivationFunctionType.Sigmoid,
                                     bias=mu_t[:, 0:1], scale=1.0)

                # s_part[p] = sum_d sum_u h[p,u]*A[p,u]  blockwise mul+reduce
                nblk = math.ceil(RPC / cfg.ttb)
                s_cols = cpool.tile([P, nblk], f32, tag="scols")
                for b in range(nblk):
                    lo = b * cfg.ttb
                    hi = min(RPC, lo + cfg.ttb)
                    tmp = evpool.tile([P, cfg.ttb], f32, tag="tmp")
                    nc.vector.tensor_tensor(
                        out=tmp[:, : hi - lo],
                        in0=acc1[:, lo:hi], in1=acc2[:, lo:hi],
                        op=mybir.AluOpType.mult)
                    nc.vector.tensor_reduce(
                        s_cols[:, b:b + 1], tmp[:, : hi - lo],
                        axis=mybir.AxisListType.X, op=mybir.AluOpType.add)
            s_t = cpool.tile([P, 1], f32, tag="sfin")
            nc.vector.tensor_reduce(s_t[:], s_cols[:], axis=mybir.AxisListType.X,
                                    op=mybir.AluOpType.add)
            nc.sync.dma_start(s_out[:], s_t[:])

    nc.compile()
    return nc


# ---------------------------------------------------------------- host driver


NPDT = {"f32": np.float32, "bf16": ml_dtypes.bfloat16, "f16": np.float16,
        "f8e4": ml_dtypes.float8_e4m3}


def make_in_maps(cfg: Cfg, ph1: PhasePlan, ph2: PhasePlan, v, w, mu):
    p1np, p2np = NPDT[cfg.p1dt], NPDT[cfg.p2dt]
    v_lo = np.ascontiguousarray(v[:cfg.half].astype(p1np))
    v_hi = np.ascontiguousarray(v[cfg.half:].astype(p1np))
    w_lo = np.ascontiguousarray(w[:cfg.half].astype(p2np))
    w_hi = np.ascontiguousarray(w[cfg.half:].astype(p2np))
    mu_col = np.broadcast_to(mu.reshape(-1)[:, None], (128, 1)).astype(np.float32)
    mu_col = np.ascontiguousarray(mu_col)
    in_maps = []
    for c in range(cfg.ncores):
        m = {
            "v_lo": v_lo, "v_hi": v_hi, "w_lo": w_lo, "w_hi": w_hi,
            "mu_col": mu_col,
            "idx1": ph1.idx_dram[c], "idx2": ph2.idx_dram[c],
        }
        if cfg.swap_ops:
            m["mu_row"] = np.ascontiguousarray(
                np.broadcast_to(mu.reshape(1, -1), (128, mu.size)).astype(
                    np.float32))
        if cfg.host_p:
            m["p1"] = ph1.p_dram[c]
            m["p2"] = ph2.p_dram[c]
        else:
            m.update({"wgt1": ph1.wgt_dram[c], "rl1": ph1.rl_dram[c],
                      "wgt2": ph2.wgt_dram[c], "rl2": ph2.rl_dram[c]})
        in_maps.append(m)
    return in_maps


_plan_cache = {}


def _fp(*arrs):
    out = []
    for a in arrs:
        a = np.asarray(a)
        v = a.reshape(-1)[:: max(1, a.size // 4096)]
        out.append((a.shape, str(a.dtype), float(np.float64(v.sum())),
                    float(np.float64((v[1::2] if v.size > 2 else v).sum()))))
    return tuple(out)


def prepare(cfg: Cfg, ij, r, i, j):
    key = (repr(cfg), _fp(ij, r, i, j))
    if key in _plan_cache:
        return _plan_cache[key]
    pnp1 = NPDT[cfg.psdt or cfg.p1dt] if cfg.host_p else None
    pnp2 = NPDT[cfg.psdt or cfg.p2dt] if cfg.host_p else None
    rows1, rows2 = np.asarray(ij[0], np.int64), np.asarray(i, np.int64)
    if cfg.balance:
        rowmap = balance_rows(cfg, rows1, ij[1], rows2, j)
        rows1, rows2 = rowmap[rows1], rowmap[rows2]
    ph1 = plan_phase(cfg, rows1, ij[1], r, pnp1)
    ph2 = plan_phase(cfg, rows2, j, np.ones(len(i), np.float32), pnp2)
    _plan_cache.clear()
    _plan_cache[key] = (ph1, ph2)
    return ph1, ph2


_prog_cache = {}


def kernel(ij, r, m, i, j, v, mu, w, b, cfg: Cfg = FULL, _return_parts=False,
           _run_kwargs=None):
    from concourse.bass_utils import run_bass_kernel_spmd

    ij = np.asarray(ij)
    r = np.asarray(r, np.float32)
    i = np.asarray(i)
    j = np.asarray(j)
    v = np.asarray(v, np.float32)
    w = np.asarray(w, np.float32)
    mu = np.asarray(mu, np.float32)
    b = np.asarray(b, np.float32)
    assert int(m) == cfg.M

    ph1, ph2 = prepare(cfg, ij, r, i, j)
    key = (cfg.M, cfg.N, ph1.total_chunks, ph2.total_chunks,
           tuple(ph1.groups), tuple(ph2.groups))
    if key not in _prog_cache:
        _prog_cache.clear()
        _prog_cache[key] = build_program(cfg, ph1, ph2)
    nc = _prog_cache[key]

    in_maps = make_in_maps(cfg, ph1, ph2, v, w, mu)
    res = run_bass_kernel_spmd(nc, in_maps, list(range(cfg.ncores)),
                               **(_run_kwargs or {}))
    parts = [res.results[c]["s_out"] for c in range(cfg.ncores)]
    s = np.float32(sum(np.asarray(p, np.float64).sum() for p in parts))
    out = s + b[j]
    if _return_parts:
        return out, res
    return out



# revision 47
# speedup vs baseline: 1.4107x; 1.4107x over previous
"""AutoRec forward kernel for Trainium2, 8-core SPMD.

Math (see reference):
    agg = segment_sum(r[:,None] * v[cols], rows, m)     # sparse (m,n) @ v
    h   = sigmoid(agg + mu)                             # (M, D)
    s   = sum(h[i] * w[j])                              # global scalar over E pairs
    out = s + b[j]                                      # (E,)

Device strategy (per core, users sharded):
  Each core owns RPC = 6272 rows (users). Both heavy stages are instances of
  one primitive: "gather rows from a replicated table, weight them, and
  segment-sum into a local per-row accumulator":
    phase 1: table=v (bf16), weights=r,     rows=ij[0], cols=ij[1] -> aggT
    phase 2: table=w (f32),  weights=1.0,   rows=i,     cols=j     -> aT
          (sum_e h[i_e] * w[j_e] = sum_u h[u] . A[u],  A[u] = sum_{i_e=u} w[j_e])
  The segment-sum runs on the tensor engine: for each chunk of 128 edges a
  host-built one-hot P[e, wrow] = weight_e * (local_row_e == wrow), streamed
  via HWDGE at fp8e4, is the STATIONARY operand (fp8 halves LoadStationary
  bytes - LS is byte-bound on trn2); the gathered rows (f16, SWDGE
  dma_gather) are the moving operand; psum accumulates agg[wrow, d] over a
  128-row window. Tables are split in two 25000-row halves (dma_gather
  indices are int16). Groups are ordered window-major (snake over halves) so
  each window's psum spans its two adjacent half-groups; at window close,
  phase 1 adds the replicated mu row on DVE and folds sigmoid(psum) into the
  h accumulator on ACT straight from psum, and phase 2 folds
  s_cols[:, w] = sum_d h_win * psum_win on DVE - no second accumulator and
  no serial tail. Rows are remapped to (core, window, slot) by a 4-D LPT
  (phase x table-half degree vectors) so per-group counts are near-equal
  across cores, tightening the shared SPMD chunk schedule (all 8 cores run
  one program; schedule = max count per group over cores). Full wrapped idx
  arrays are preloaded into SBUF once, killing per-call idx DMAs. The host
  sums the 8 s_out partials and broadcasts s + b[j] (trivial O(E) numpy).
"""

import math
from dataclasses import dataclass, field

import ml_dtypes
import numpy as np

# ---------------------------------------------------------------- config

CHUNK = 128  # edges per matmul (contraction = partition dim)
IDX_WRAP = 16  # dma_gather index wrap


@dataclass
class Cfg:
    M: int = 50000          # users (rows of spmm)
    dma_scratch: int = 16384  # SWDGE descriptor carveout (bytes)
    N: int = 50000          # items (table rows)
    D: int = 128            # feature dim (must be 128)
    ncores: int = 8
    rpc: int = 6272         # rows per core (multiple of window)
    window: int = 128       # psum row-window
    half: int = 25000       # table split (int16 index limit)
    call_chunks: int = 8    # chunks per dma_gather call (HW SWDGE ring caps ~1024 idxs/call)
    p1dt: str = "f16"       # value dtype of phase-1 gathers / one-hot
    p2dt: str = "f16"       # value dtype of phase-2 gathers / one-hot
    psdt: str = ""          # P stream dtype override ("" = same as gathers)
    wmajor: bool = False    # window-major group order + fused per-window fold
    ttb: int = 512          # block size of the final fused mul-reduce
    queues: int = 4         # SWDGE queues to round-robin gather calls over
    balance: bool = False   # degree-balanced row->(core,window,slot) remap
    host_p: bool = True     # precompute one-hot P on host, stream via HWDGE
    p_dve_frac: float = 0.0  # fraction of chunks whose P is DVE-built instead
    swap_ops: bool = False  # P as stationary (fp8 LS), gathered tile as moving
    sblk: int = 0           # >0: superblock size (windows per half-major block)
    gbufs: int = 8          # gather-tile pool bufs
    pbufs: int = 10         # one-hot P pool bufs
    ibufs: int = 10         # idx pool bufs

    @property
    def nwin(self):
        return self.rpc // self.window

    def __post_init__(self):
        assert self.rpc % self.window == 0
        assert self.rpc * self.ncores >= self.M
        assert self.N <= 2 * self.half and self.half <= 32767
        assert self.D == 128


FULL = Cfg(wmajor=True, host_p=True, balance=True, call_chunks=8,
           dma_scratch=49152, gbufs=8, ibufs=2, pbufs=16, psdt="f8e4",
           swap_ops=True, p_dve_frac=0.25, sblk=8)

# ---------------------------------------------------------------- host plan


@dataclass
class PhasePlan:
    groups: list          # [(hf, win, n_chunks)] in stream order (hf-major)
    calls: list           # [(hf, chunk_start, n_chunks)]
    total_chunks: int
    # per-core packed arrays
    idx_dram: list        # [ncores] int16 [128, total_chunks*8]
    wgt_dram: list        # [ncores] [128, total_chunks]
    rl_dram: list         # [ncores] [128, total_chunks]
    p_dram: list = None   # [ncores] pdt [128, total_chunks*W] host one-hot


def _wrap_idxs(ii: np.ndarray) -> np.ndarray:
    """[n] -> [128, n/16] wrapped (t -> (t%16, t//16)), replicated x8."""
    n = len(ii)
    a = ii.reshape(n // IDX_WRAP, IDX_WRAP).T
    return np.tile(a, (8, 1))


def balance_rows(cfg: Cfg, rows1, cols1, rows2, cols2) -> np.ndarray:
    """4-D LPT: pack rows into (core, window) buckets of 128 slots balancing
    each row's (phase x table-half) degree vector, so per-(half,win) counts
    are near-equal across cores -> tight shared chunk schedule.
    Returns rowmap [M]: row -> virtual row id (core*rpc + win*128 + slot)."""
    nb = cfg.ncores * cfg.nwin
    deg = np.zeros((cfg.M, 4), np.int64)
    for k, (rr, cc) in enumerate(((rows1, cols1), (rows2, cols2))):
        rr = np.asarray(rr, np.int64)
        hf = (np.asarray(cc, np.int64) >= cfg.half).astype(np.int64)
        np.add.at(deg, (rr, 2 * k + hf), 1)
    order = np.argsort(-deg.sum(1), kind="stable")
    load = np.zeros((nb, 4), np.float64)
    fill = np.zeros(nb, np.int64)
    full_pen = np.zeros(nb, np.float64)
    rowmap = np.zeros(cfg.M, np.int64)
    for row in order:
        cost = (load + deg[row]).max(axis=1) + full_pen
        b = int(np.argmin(cost))
        rowmap[row] = b * cfg.window + fill[b]
        fill[b] += 1
        load[b] += deg[row]
        if fill[b] >= cfg.window:
            full_pen[b] = np.inf
    return rowmap


def plan_phase(cfg: Cfg, rows, cols, wgts, pnp=None) -> PhasePlan:
    rows = np.asarray(rows, np.int64)
    cols = np.asarray(cols, np.int64)
    wgts = np.asarray(wgts, np.float32)
    nwin, ncores, Wd = cfg.nwin, cfg.ncores, cfg.window

    core = rows // cfg.rpc
    local = rows - core * cfg.rpc
    win = local // Wd
    rl = (local - win * Wd).astype(np.float32)
    hf = (cols >= cfg.half).astype(np.int64)
    idx16 = (cols - hf * cfg.half).astype(np.int16)

    key = (core * 2 + hf) * nwin + win
    counts = np.bincount(key, minlength=ncores * 2 * nwin).reshape(ncores, 2, nwin)
    nch = -(-counts.max(axis=0) // CHUNK)  # [2, nwin] ceil
    groups = []
    gbase = np.zeros((2, nwin), np.int64)
    acc = 0
    if cfg.wmajor and cfg.sblk > 0:
        # superblocks: K windows half-major, snaking halves across blocks so
        # same-half runs merge at block boundaries; a window's two groups sit
        # K apart -> run_phase keeps K psums open
        order = []
        blocks = [list(range(b, min(b + cfg.sblk, nwin)))
                  for b in range(0, nwin, cfg.sblk)]
        for bi, ws in enumerate(blocks):
            hs = (0, 1) if bi % 2 == 0 else (1, 0)
            for h in hs:
                for w in ws:
                    order.append((h, w))
    elif cfg.wmajor:
        # window-major, snake over halves so adjacent same-half runs merge
        order = []
        for w in range(nwin):
            hs = (0, 1) if w % 2 == 0 else (1, 0)
            for h in hs:
                order.append((h, w))
    else:
        order = [(h, w) for h in range(2) for w in range(nwin)]
    for h, w in order:
        n = int(nch[h, w])
        if n == 0:
            continue
        groups.append((h, w, n))
        gbase[h, w] = acc
        acc += n
    total_chunks = acc

    # gather calls: split maximal same-half chunk runs into spans of
    # call_chunks (a call reads one table half)
    calls = []
    runs = []
    for g, (h, w, n) in enumerate(groups):
        if runs and runs[-1][0] == h:
            runs[-1][2] += n
        else:
            runs.append([h, sum(nn for (_, _, nn) in groups[:g]), n])
    for h, start, n in runs:
        off = start
        while off < start + n:
            k = min(cfg.call_chunks, start + n - off)
            calls.append((h, off, k))
            off += k
    assert sum(n for (_, _, n) in calls) == total_chunks

    idx_l, wgt_l, rl_l, p_l = [], [], [], []
    for c in range(ncores):
        mask = core == c
        eh, ew = hf[mask], win[mask]
        erl, ei, ewgt = rl[mask], idx16[mask], wgts[mask]
        order = np.lexsort((ew, eh))
        eh, ew, erl, ei, ewgt = (a[order] for a in (eh, ew, erl, ei, ewgt))
        gid = eh * nwin + ew
        # rank within each (hf,win) run of the sorted list
        if len(gid):
            first = np.r_[True, gid[1:] != gid[:-1]]
            run_start = np.maximum.accumulate(np.where(first, np.arange(len(gid)), 0))
            rank = np.arange(len(gid)) - run_start
        else:
            rank = np.zeros(0, np.int64)
        pos = gbase[eh, ew] * CHUNK + rank
        idx_full = np.zeros(total_chunks * CHUNK, np.int16)
        wgt_full = np.zeros(total_chunks * CHUNK, np.float32)
        rl_full = np.zeros(total_chunks * CHUNK, np.float32)
        idx_full[pos] = ei
        wgt_full[pos] = ewgt
        rl_full[pos] = erl

        # wrap idx per call
        parts = []
        for (_h, c0, n) in calls:
            parts.append(_wrap_idxs(idx_full[c0 * CHUNK:(c0 + n) * CHUNK]))
        idx_l.append(np.concatenate(parts, axis=1))
        wgt_l.append(wgt_full.reshape(-1, CHUNK).T.copy())
        rl_l.append(rl_full.reshape(-1, CHUNK).T.copy())
        if pnp is not None:
            # host one-hot: P[p, chunk, x] = wgt * (rl == x), edge = chunk*128+p
            parr = np.zeros((CHUNK, total_chunks, Wd), pnp)
            epos = np.arange(total_chunks * CHUNK)
            parr[epos % CHUNK, epos // CHUNK, rl_full.astype(np.int64)] = wgt_full
            p_l.append(np.ascontiguousarray(parr.reshape(CHUNK, -1)))

    return PhasePlan(groups, calls, total_chunks, idx_l, wgt_l, rl_l,
                     p_l if pnp is not None else None)


# ---------------------------------------------------------------- device build


def build_program(cfg: Cfg, ph1: PhasePlan, ph2: PhasePlan):
    import concourse.bacc as bacc
    import concourse.bass as bass
    import concourse.mybir as mybir
    import concourse.tile as tile

    f32 = mybir.dt.float32
    i16 = mybir.dt.int16
    DTMAP = {"f32": f32, "bf16": mybir.dt.bfloat16, "f16": mybir.dt.float16,
             "f8e4": mybir.dt.float8e4}
    p1dt, p2dt = DTMAP[cfg.p1dt], DTMAP[cfg.p2dt]
    psdt = DTMAP[cfg.psdt] if cfg.psdt else None
    P, Wd, RPC = 128, cfg.window, cfg.rpc
    n_hi = cfg.N - cfg.half

    nc = bacc.Bacc("TRN2", target_bir_lowering=False, debug=False,
                   dynamic_dma_scratch_size=cfg.dma_scratch,
                   num_swdge_queues=cfg.queues)

    v_lo = nc.dram_tensor("v_lo", [cfg.half, cfg.D], p1dt, kind="ExternalInput")
    v_hi = nc.dram_tensor("v_hi", [n_hi, cfg.D], p1dt, kind="ExternalInput")
    w_lo = nc.dram_tensor("w_lo", [cfg.half, cfg.D], p2dt, kind="ExternalInput")
    w_hi = nc.dram_tensor("w_hi", [n_hi, cfg.D], p2dt, kind="ExternalInput")
    mu_c = nc.dram_tensor("mu_col", [P, 1], f32, kind="ExternalInput")
    if cfg.swap_ops:
        mu_r = nc.dram_tensor("mu_row", [P, cfg.D], f32, kind="ExternalInput")

    need_build = (not cfg.host_p) or cfg.p_dve_frac > 0

    def phase_params(tag, pl: PhasePlan, pdt):
        idx = nc.dram_tensor(f"idx{tag}", [P, pl.total_chunks * 8], i16,
                             kind="ExternalInput")
        pd = wgt = rl = None
        if cfg.host_p:
            pd = nc.dram_tensor(f"p{tag}", [P, pl.total_chunks * Wd],
                                psdt or pdt, kind="ExternalInput")
        if need_build:
            wgt = nc.dram_tensor(f"wgt{tag}", [P, pl.total_chunks], f32,
                                 kind="ExternalInput")
            rl = nc.dram_tensor(f"rl{tag}", [P, pl.total_chunks], f32,
                                kind="ExternalInput")
        return idx, pd, wgt, rl

    idx1, p1, wgt1, rl1 = phase_params(1, ph1, p1dt)
    idx2, p2, wgt2, rl2 = phase_params(2, ph2, p2dt)
    s_out = nc.dram_tensor("s_out", [P, 1], f32, kind="ExternalOutput")

    with tile.TileContext(nc) as tc:
        with (
            tc.tile_pool(name="const", bufs=1) as cpool,
            tc.tile_pool(name="idxp", bufs=cfg.ibufs) as ipool,
            tc.tile_pool(name="g1", bufs=cfg.gbufs) as g1pool,
            tc.tile_pool(name="g2", bufs=cfg.gbufs) as g2pool,
            tc.tile_pool(name="pp", bufs=cfg.pbufs) as ppool,
            tc.tile_pool(name="ev", bufs=4) as evpool,
            tc.tile_pool(name="psum", bufs=8, space="PSUM") as pspool,
        ):
            # constants
            mu_t = cpool.tile([P, 1], f32, tag="mu")
            nc.sync.dma_start(mu_t[:], mu_c[:])
            if cfg.swap_ops:
                mu_rt = cpool.tile([P, cfg.D], f32, tag="mur")
                nc.sync.dma_start(mu_rt[:], mu_r[:])
            if need_build:
                iota1 = cpool.tile([P, Wd], p1dt, tag="iota1")
                iota2 = cpool.tile([P, Wd], p2dt, tag="iota2")
                nc.gpsimd.iota(iota1[:], pattern=[[1, Wd]], base=0,
                               channel_multiplier=0,
                               allow_small_or_imprecise_dtypes=True)
                nc.gpsimd.iota(iota2[:], pattern=[[1, Wd]], base=0,
                               channel_multiplier=0,
                               allow_small_or_imprecise_dtypes=True)
            else:
                iota1 = iota2 = None

            acc1 = cpool.tile([P, RPC], f32, tag="acc1")
            nc.vector.memset(acc1[:], 0.0)
            if not cfg.wmajor:
                acc2 = cpool.tile([P, RPC], f32, tag="acc2")
                nc.vector.memset(acc2[:], 0.0)
            else:
                acc2 = None

            if need_build:
                wg1_t = cpool.tile([P, ph1.total_chunks], f32, tag="wg1")
                rl1_t = cpool.tile([P, ph1.total_chunks], f32, tag="rl1")
                wg2_t = cpool.tile([P, ph2.total_chunks], f32, tag="wg2")
                rl2_t = cpool.tile([P, ph2.total_chunks], f32, tag="rl2")
                nc.sync.dma_start(wg1_t[:], wgt1[:])
                nc.sync.dma_start(rl1_t[:], rl1[:])
                nc.sync.dma_start(wg2_t[:], wgt2[:])
                nc.sync.dma_start(rl2_t[:], rl2[:])
            else:
                wg1_t = rl1_t = wg2_t = rl2_t = None

            qcount = [0]

            # preload full wrapped-idx arrays once (kills per-call idx DMAs)
            idx1_t = cpool.tile([P, ph1.total_chunks * 8], i16, tag="idx1t")
            idx2_t = cpool.tile([P, ph2.total_chunks * 8], i16, tag="idx2t")
            nc.sync.dma_start(idx1_t[:], idx1[:])
            nc.sync.dma_start(idx2_t[:], idx2[:])

            def run_phase(pl: PhasePlan, tabs, idx_t, p_dram, wg_t, rl_t,
                          fold, gpool, pdt, io_t):
                # map chunk id -> (win, first?, last?); first/last span the
                # whole window (wmajor: its adjacent half-groups) or one group
                chunk_group = {}
                base = 0
                if cfg.wmajor:
                    # chunks of window w (possibly split across separated
                    # groups): first/last flags span the whole window
                    wchunks = {}
                    for (h, w, n) in pl.groups:
                        wchunks.setdefault(w, []).extend(
                            range(base, base + n))
                        base += n
                    for w, cs in wchunks.items():
                        for idx, cid in enumerate(cs):
                            chunk_group[cid] = (w, idx == 0,
                                                idx == len(cs) - 1)
                else:
                    for (h, w, n) in pl.groups:
                        for k in range(n):
                            chunk_group[base + k] = (w, k == 0, k == n - 1)
                        base += n
                # consume calls in order, carrying the open psum group
                open_ps = {}
                for (h, c0, n) in pl.calls:
                    gt = gpool.tile([P, cfg.call_chunks, cfg.D], pdt, tag="g")
                    nidx = n * CHUNK
                    nc.gpsimd.dma_gather(
                        gt[:, :n, :], tabs[h][:], idx_t[:, c0 * 8:(c0 + n) * 8],
                        num_idxs=nidx, num_idxs_reg=nidx, elem_size=cfg.D,
                        queue_num=qcount[0] % cfg.queues,
                    )
                    qcount[0] += 1
                    if not cfg.host_p:
                        k_dve = n
                    else:
                        k_dve = int(round(cfg.p_dve_frac * n))
                    if n > k_dve:
                        pc_t = ppool.tile([P, cfg.call_chunks * Wd],
                                          psdt or pdt, tag="pc")
                        nc.sync.dma_start(
                            pc_t[:, : (n - k_dve) * Wd],
                            p_dram[:, (c0 + k_dve) * Wd:(c0 + n) * Wd])
                    for k in range(n):
                        cid = c0 + k
                        w, first, last = chunk_group[cid]
                        if first:
                            ps_new = pspool.tile([P, Wd], mybir.dt.float32,
                                                 tag="ps")
                            open_ps[w] = ps_new
                        if k >= k_dve:
                            p_ap = pc_t[:, (k - k_dve) * Wd:(k - k_dve + 1) * Wd]
                        else:
                            p_t = ppool.tile([P, Wd], psdt or pdt, tag="p")
                            nc.vector.tensor_scalar(
                                out=p_t[:], in0=io_t[:],
                                scalar1=rl_t[:, cid:cid + 1],
                                scalar2=wg_t[:, cid:cid + 1],
                                op0=mybir.AluOpType.is_equal,
                                op1=mybir.AluOpType.mult,
                            )
                            p_ap = p_t[:]
                        ps = open_ps[w]
                        if cfg.swap_ops:
                            nc.tensor.matmul(ps[:], p_ap, gt[:, k, :],
                                             start=first, stop=last)
                        else:
                            nc.tensor.matmul(ps[:], gt[:, k, :], p_ap,
                                             start=first, stop=last)
                        if last:
                            fold(w, ps)
                            del open_ps[w]

            if cfg.wmajor:
                s_cols = cpool.tile([P, cfg.nwin], f32, tag="scols")
                nc.vector.memset(s_cols[:], 0.0)

                def fold1(w, ps):
                    # h_win = sigmoid(agg_win + mu), psum -> acc1 slice
                    if cfg.swap_ops:
                        # [u, d] layout: mu varies along free dim; add the
                        # replicated mu row on DVE, then plain sigmoid on ACT
                        nc.vector.tensor_tensor(
                            out=ps[:], in0=ps[:], in1=mu_rt[:],
                            op=mybir.AluOpType.add)
                        nc.scalar.activation(
                            acc1[:, w * Wd:(w + 1) * Wd], ps[:],
                            mybir.ActivationFunctionType.Sigmoid,
                            bias=0.0, scale=1.0)
                    else:
                        nc.scalar.activation(
                            acc1[:, w * Wd:(w + 1) * Wd], ps[:],
                            mybir.ActivationFunctionType.Sigmoid,
                            bias=mu_t[:, 0:1], scale=1.0)

                def fold2(w, ps):
                    # s_cols[:, w] = sum_u h_win * A_win  (A straight from psum)
                    tmp = evpool.tile([P, Wd], f32, tag="tmp")
                    nc.vector.tensor_tensor(
                        out=tmp[:], in0=acc1[:, w * Wd:(w + 1) * Wd],
                        in1=ps[:], op=mybir.AluOpType.mult)
                    nc.vector.tensor_reduce(
                        s_cols[:, w:w + 1], tmp[:],
                        axis=mybir.AxisListType.X, op=mybir.AluOpType.add)

                run_phase(ph1, (v_lo, v_hi), idx1_t, p1, wg1_t, rl1_t,
                          fold1, g1pool, p1dt, iota1)
                run_phase(ph2, (w_lo, w_hi), idx2_t, p2, wg2_t, rl2_t,
                          fold2, g2pool, p2dt, iota2)
            else:
                def mkfold(acc):
                    def fold(w, ps):
                        sl = acc[:, w * Wd:(w + 1) * Wd]
                        nc.vector.tensor_tensor(
                            out=sl, in0=sl, in1=ps[:],
                            op=mybir.AluOpType.add)
                    return fold

                run_phase(ph1, (v_lo, v_hi), idx1_t, p1, wg1_t, rl1_t,
                          mkfold(acc1), g1pool, p1dt, iota1)
                run_phase(ph2, (w_lo, w_hi), idx2_t, p2, wg2_t, rl2_t,
                          mkfold(acc2), g2pool, p2dt, iota2)

                # h = sigmoid(aggT + mu)  (in place on acc1)
                nc.scalar.activation(acc1[:], acc1[:],
                                     mybir.ActivationFunctionType.Sigmoid,
                                     bias=mu_t[:, 0:1], scale=1.0)

                # s_part[p] = sum_d sum_u h[p,u]*A[p,u]  blockwise mul+reduce
                nblk = math.ceil(RPC / cfg.ttb)
                s_cols = cpool.tile([P, nblk], f32, tag="scols")
                for b in range(nblk):
                    lo = b * cfg.ttb
                    hi = min(RPC, lo + cfg.ttb)
                    tmp = evpool.tile([P, cfg.ttb], f32, tag="tmp")
                    nc.vector.tensor_tensor(
                        out=tmp[:, : hi - lo],
                        in0=acc1[:, lo:hi], in1=acc2[:, lo:hi],
                        op=mybir.AluOpType.mult)
                    nc.vector.tensor_reduce(
                        s_cols[:, b:b + 1], tmp[:, : hi - lo],
                        axis=mybir.AxisListType.X, op=mybir.AluOpType.add)
            s_t = cpool.tile([P, 1], f32, tag="sfin")
            nc.vector.tensor_reduce(s_t[:], s_cols[:], axis=mybir.AxisListType.X,
                                    op=mybir.AluOpType.add)
            nc.sync.dma_start(s_out[:], s_t[:])

    nc.compile()
    return nc


# ---------------------------------------------------------------- host driver


NPDT = {"f32": np.float32, "bf16": ml_dtypes.bfloat16, "f16": np.float16,
        "f8e4": ml_dtypes.float8_e4m3}


def make_in_maps(cfg: Cfg, ph1: PhasePlan, ph2: PhasePlan, v, w, mu):
    p1np, p2np = NPDT[cfg.p1dt], NPDT[cfg.p2dt]
    v_lo = np.ascontiguousarray(v[:cfg.half].astype(p1np))
    v_hi = np.ascontiguousarray(v[cfg.half:].astype(p1np))
    w_lo = np.ascontiguousarray(w[:cfg.half].astype(p2np))
    w_hi = np.ascontiguousarray(w[cfg.half:].astype(p2np))
    mu_col = np.broadcast_to(mu.reshape(-1)[:, None], (128, 1)).astype(np.float32)
    mu_col = np.ascontiguousarray(mu_col)
    in_maps = []
    for c in range(cfg.ncores):
        m = {
            "v_lo": v_lo, "v_hi": v_hi, "w_lo": w_lo, "w_hi": w_hi,
            "mu_col": mu_col,
            "idx1": ph1.idx_dram[c], "idx2": ph2.idx_dram[c],
        }
        if cfg.swap_ops:
            m["mu_row"] = np.ascontiguousarray(
                np.broadcast_to(mu.reshape(1, -1), (128, mu.size)).astype(
                    np.float32))
        if cfg.host_p:
            m["p1"] = ph1.p_dram[c]
            m["p2"] = ph2.p_dram[c]
        if (not cfg.host_p) or cfg.p_dve_frac > 0:
            m.update({"wgt1": ph1.wgt_dram[c], "rl1": ph1.rl_dram[c],
                      "wgt2": ph2.wgt_dram[c], "rl2": ph2.rl_dram[c]})
        in_maps.append(m)
    return in_maps


_plan_cache = {}


def _fp(*arrs):
    out = []
    for a in arrs:
        a = np.asarray(a)
        v = a.reshape(-1)[:: max(1, a.size // 4096)]
        out.append((a.shape, str(a.dtype), float(np.float64(v.sum())),
                    float(np.float64((v[1::2] if v.size > 2 else v).sum()))))
    return tuple(out)


def prepare(cfg: Cfg, ij, r, i, j):
    key = (repr(cfg), _fp(ij, r, i, j))
    if key in _plan_cache:
        return _plan_cache[key]
    pnp1 = NPDT[cfg.psdt or cfg.p1dt] if cfg.host_p else None
    pnp2 = NPDT[cfg.psdt or cfg.p2dt] if cfg.host_p else None
    rows1, rows2 = np.asarray(ij[0], np.int64), np.asarray(i, np.int64)
    if cfg.balance:
        rowmap = balance_rows(cfg, rows1, ij[1], rows2, j)
        rows1, rows2 = rowmap[rows1], rowmap[rows2]
    ph1 = plan_phase(cfg, rows1, ij[1], r, pnp1)
    ph2 = plan_phase(cfg, rows2, j, np.ones(len(i), np.float32), pnp2)
    _plan_cache.clear()
    _plan_cache[key] = (ph1, ph2)
    return ph1, ph2


_prog_cache = {}


def kernel(ij, r, m, i, j, v, mu, w, b, cfg: Cfg = FULL, _return_parts=False,
           _run_kwargs=None):
    from concourse.bass_utils import run_bass_kernel_spmd

    ij = np.asarray(ij)
    r = np.asarray(r, np.float32)
    i = np.asarray(i)
    j = np.asarray(j)
    v = np.asarray(v, np.float32)
    w = np.asarray(w, np.float32)
    mu = np.asarray(mu, np.float32)
    b = np.asarray(b, np.float32)
    assert int(m) == cfg.M

    ph1, ph2 = prepare(cfg, ij, r, i, j)
    key = (cfg.M, cfg.N, ph1.total_chunks, ph2.total_chunks,
           tuple(ph1.groups), tuple(ph2.groups))
    if key not in _prog_cache:
        _prog_cache.clear()
        _prog_cache[key] = build_program(cfg, ph1, ph2)
    nc = _prog_cache[key]

    in_maps = make_in_maps(cfg, ph1, ph2, v, w, mu)
    res = run_bass_kernel_spmd(nc, in_maps, list(range(cfg.ncores)),
                               **(_run_kwargs or {}))
    parts = [res.results[c]["s_out"] for c in range(cfg.ncores)]
    s = np.float32(sum(np.asarray(p, np.float64).sum() for p in parts))
    out = s + b[j]
    if _return_parts:
        return out, res
    return out

